# revision 1
# baseline (speedup 1.0000x reference)
"""Quantum angle-encoder state-vector kernel for Trainium2 (8 NeuronCores).

For each batch row b and qubit q the gate rz*ry applied to |0> contributes a
2-vector col0 = cos(ry/2)e^{-i rz/2}, col1 = sin(ry/2)e^{+i rz/2}; the output
state is the Kronecker product over 16 qubits (qubit 0 = MSB), [B, 2^16] c64.

Per core (32 batch rows, pure data parallel over 8 cores):
  * v = v_hi (x) v_lo with v_hi/v_lo the 8-qubit half-products (length 256).
    Both halves are built in POLAR form, stacked on 64 partitions:
      - phases are additive -> ONE K=16 TensorE matmul against a constant
        0/1 selection matrix computes all 256 phase sums per row;
      - magnitudes multiply -> 7-step scalar-broadcast chain on ScalarE;
      - range-reduce theta to [-pi,pi] (Sin LUT domain) via an
        f32->i32->f32 rounding cast, then m*cos / m*sin.
  * The 256x256 outer product is a K=12 bf16 matmul per (b, i-chunk):
    each fp32 factor is split into 3 bf16 terms (24-bit exact); rhs columns
    are pre-interleaved so PSUM comes out in complex64 memory order.
  * PSUM -> SBUF copy (ScalarE/VectorE), SBUF -> HBM DMA, issue spread over
    SP/ACT sequencers (each dma_start costs ~0.6us on its sequencer).

Notes for this toolchain: walrus here encodes at most ONE semaphore wait per
instruction -- _legalize_single_wait() hoists extra Tile-emitted waits into
standalone EventSemaphore instructions. Output per core [32,2,128,512] f32 ==
[32, 65536] complex64 (viewed on host).
"""

import numpy as np

import concourse.bass as bass
import concourse.mybir as mybir
import concourse.tile as tile
from concourse.bass_utils import run_bass_kernel_spmd

N_CORES = 8
B, Q = 256, 16
BC = B // N_CORES  # batch rows per core
HQ = Q // 2  # qubits per half
HL = 1 << HQ  # 256: length of each half-product
F32 = mybir.dt.float32
BF16 = mybir.dt.bfloat16
I32 = mybir.dt.int32
PI_HALF = float(np.pi / 2)

_AF = mybir.ActivationFunctionType
_OP = mybir.AluOpType


def _emit_mag_chain(nc, pool, MAG0, MAG1):
    """Magnitude half of the stacked Kronecker product: per step multiply by
    a per-partition scalar on the ScalarEngine only. [2*BC, HL] result."""
    P2 = 2 * BC
    mA = pool.tile([P2, HL], F32, tag="st_mA")
    mB = pool.tile([P2, HL], F32, tag="st_mB")
    q = HQ - 1
    nc.scalar.copy(mA[:, 0:1], MAG0[:, q : q + 1])
    nc.scalar.copy(mA[:, 1:2], MAG1[:, q : q + 1])
    cur_m, nxt_m = mA, mB
    L = 2
    for q in range(HQ - 2, -1, -1):
        for t, MG in enumerate((MAG0, MAG1)):
            nc.scalar.mul(nxt_m[:, t * L : (t + 1) * L], cur_m[:, 0:L], MG[:, q : q + 1])
        cur_m, nxt_m = nxt_m, cur_m
        L *= 2
    return cur_m


def _theta_to_cartesian(nc, pool, theta, cur_m, pih):
    """Range-reduce theta (PSUM) into [-pi, pi], take sin/cos, multiply by
    the magnitudes. Returns (vr, vi) [2*BC, HL] f32."""
    P2 = 2 * BC
    INV2PI = float(1.0 / (2.0 * np.pi))
    TWO_PI_HI = float(np.float32(2.0 * np.pi))
    TWO_PI_LO = float(2.0 * np.pi - float(np.float32(2.0 * np.pi)))

    def reduce(src, tagp):
        t1 = pool.tile([P2, HL], F32, tag=f"{tagp}_t1")
        nc.vector.tensor_scalar_mul(t1[:], src, INV2PI)
        ni = pool.tile([P2, HL], I32, tag=f"{tagp}_ni")
        nc.vector.tensor_copy(ni[:], t1[:])
        nf = pool.tile([P2, HL], F32, tag=f"{tagp}_nf")
        nc.vector.tensor_copy(nf[:], ni[:])
        r1 = pool.tile([P2, HL], F32, tag=f"{tagp}_r1")
        nc.vector.scalar_tensor_tensor(
            r1[:], nf[:], -TWO_PI_HI, src, op0=_OP.mult, op1=_OP.add
        )
        red = pool.tile([P2, HL], F32, tag=f"{tagp}_red")
        nc.vector.scalar_tensor_tensor(
            red[:], nf[:], -TWO_PI_LO, r1[:], op0=_OP.mult, op1=_OP.add
        )
        return red

    red_s = reduce(theta, "rs")
    thc = pool.tile([P2, HL], F32, tag="st_thc")
    nc.vector.tensor_scalar_add(thc[:], theta, PI_HALF)
    red_c = reduce(thc[:], "rc")

    cosb = pool.tile([P2, HL], F32, tag="st_cos")
    sinb = pool.tile([P2, HL], F32, tag="st_sin")
    nc.scalar.activation(cosb[:], red_c[:], _AF.Sin, scale=1.0)
    nc.scalar.activation(sinb[:], red_s[:], _AF.Sin, scale=1.0)
    vr = pool.tile([P2, HL], F32, tag="st_vr")
    vi = pool.tile([P2, HL], F32, tag="st_vi")
    nc.vector.tensor_mul(vr[:], cur_m[:], cosb[:])
    nc.vector.tensor_mul(vi[:], cur_m[:], sinb[:])
    return vr, vi


def _legalize_single_wait(nc):
    """This walrus build encodes at most one semaphore wait per instruction
    ("Too many sync wait commands" otherwise). Hoist extra waits into
    standalone EventSemaphore instructions placed immediately before — a
    sequencer-level wait gates everything after it on the same engine, so
    semantics are preserved (slightly stronger ordering)."""
    cnt = 0
    for fn in nc.m.functions:
        for blk in fn.blocks:
            out = []
            for ins in blk.instructions:
                si = ins.sync_info
                if si is not None and si.on_wait is not None and len(si.on_wait) > 1:
                    waits = list(si.on_wait)
                    for w in waits[:-1]:
                        cnt += 1
                        ev = mybir.InstEventSemaphore(
                            name=f"{ins.name}-presync-{cnt}", ins=[], outs=[]
                        )
                        ev.engine = ins.engine
                        ev.sync_info = mybir.SyncInfo(on_wait=[w], on_update=[])
                        out.append(ev)
                    ins.sync_info = mybir.SyncInfo(
                        on_wait=[waits[-1]], on_update=list(si.on_update)
                    )
                out.append(ins)
            try:
                blk.instructions = out
            except Exception:
                blk.instructions[:] = out
    return cnt


def build_bass():
    nc = bass.Bass()
    ry_d = nc.dram_tensor("ry", [BC, Q], F32, kind="ExternalInput")
    rz_d = nc.dram_tensor("rz", [BC, Q], F32, kind="ExternalInput")
    out_d = nc.dram_tensor("out", [BC, 2, 128, 512], F32, kind="ExternalOutput")

    ident_np = np.eye(2 * BC, dtype=np.float32)
    ident_d = nc.inline_tensor(ident_np, name="ident_const")
    sel_np = np.zeros((2 * HQ, HL), dtype=np.float32)
    for q in range(HQ):
        for t in range(2):
            bits = (np.arange(HL) >> (HQ - 1 - q)) & 1
            sel_np[t * HQ + q, :] = (bits == t).astype(np.float32)
    sel_d = nc.inline_tensor(sel_np, name="sel_const")

    with tile.TileContext(nc) as tc:
        with (
            tc.tile_pool(name="io", bufs=1) as io,
            tc.tile_pool(name="stage", bufs=28) as stage,
            tc.tile_pool(name="psum", bufs=6, space="PSUM") as psum,
        ):
            P2 = 2 * BC
            # Stacked angle layout [2*BC, HQ]: rows 0..BC-1 = qubits 0..7,
            # rows BC.. = qubits 8..15 (same batch rows), so the hi and lo
            # half-products advance in ONE chain over 64 partitions.
            sry = io.tile([P2, HQ], F32, tag="sry")
            srz = io.tile([P2, HQ], F32, tag="srz")
            nc.sync.dma_start(sry[0:BC, :], ry_d[:, 0:HQ])
            nc.scalar.dma_start(sry[BC:P2, :], ry_d[:, HQ:Q])
            nc.sync.dma_start(srz[0:BC, :], rz_d[:, 0:HQ])
            nc.scalar.dma_start(srz[BC:P2, :], rz_d[:, HQ:Q])

            # Per-qubit columns in polar form:
            #   col0 = cos(ry/2) * e^{-i rz/2} -> mag |cos(ry/2)|,
            #          phase -rz/2 + pi*[cos(ry/2) < 0]
            #   col1 = sin(ry/2) * e^{+i rz/2} -> mag |sin(ry/2)|,
            #          phase +rz/2 + pi*[sin(ry/2) < 0]
            pih = io.tile([P2, 1], F32, tag="pih")
            nc.vector.memset(pih[:], PI_HALF)
            c = io.tile([P2, HQ], F32, tag="c")
            s = io.tile([P2, HQ], F32, tag="s")
            nc.scalar.activation(c[:], sry[:], _AF.Sin, bias=pih[:], scale=0.5)
            nc.scalar.activation(s[:], sry[:], _AF.Sin, scale=0.5)
            MAG0 = io.tile([P2, HQ], F32, tag="MAG0")
            MAG1 = io.tile([P2, HQ], F32, tag="MAG1")
            nc.scalar.activation(MAG0[:], c[:], _AF.Abs)
            nc.scalar.activation(MAG1[:], s[:], _AF.Abs)
            hrz = io.tile([P2, HQ], F32, tag="hrz")
            nc.vector.tensor_scalar_mul(hrz[:], srz[:], 0.5)
            mkc = io.tile([P2, HQ], F32, tag="mkc")
            mks = io.tile([P2, HQ], F32, tag="mks")
            nc.vector.tensor_scalar(mkc[:], c[:], 0.0, None, op0=_OP.is_lt)
            nc.vector.tensor_scalar(mks[:], s[:], 0.0, None, op0=_OP.is_lt)
            # Phases land side by side in PHI [2*BC, 16] (cols 0..7 = phi0,
            # 8..15 = phi1); one PE transpose then one K=16 selection matmul
            # computes ALL 256 phase sums per row: SEL[(t*8+q), i] = 1 iff
            # bit q of i equals t (qubit column 0 = MSB of the half-index).
            PHI = io.tile([P2, 2 * HQ], F32, tag="PHI")
            PI = float(np.pi)
            nc.vector.scalar_tensor_tensor(
                PHI[:, 0:HQ], mkc[:], PI, hrz[:], op0=_OP.mult, op1=_OP.subtract
            )
            nc.vector.scalar_tensor_tensor(
                PHI[:, HQ : 2 * HQ], mks[:], PI, hrz[:], op0=_OP.mult, op1=_OP.add
            )
            ident = io.tile([P2, P2], F32, tag="ident")
            nc.sync.dma_start(ident[:], ident_d[:])
            sel = io.tile([2 * HQ, HL], F32, tag="sel")
            nc.sync.dma_start(sel[:], sel_d[:])
            tp = psum.tile([2 * HQ, P2], F32, tag="tpth", bufs=1)
            nc.tensor.transpose(tp[:], PHI[:], ident[:])
            vals = io.tile([2 * HQ, P2], F32, tag="vals")
            nc.vector.tensor_copy(vals[:], tp[:])
            theta = psum.tile([P2, HL], F32, tag="tpth", bufs=1)
            nc.tensor.matmul(theta[:], vals[:], sel[:], start=True, stop=True)

            cur_m = _emit_mag_chain(nc, io, MAG0, MAG1)
            st_r, st_i = _theta_to_cartesian(nc, io, theta[:], cur_m, pih)

            # fp32 matmul on PE runs at quarter rate; instead split each fp32
            # factor into 3 bf16 terms (h + m + l covers the full 24-bit
            # mantissa) and run full-rate bf16 matmuls with K=12. Products
            # (h,h),(h,m),(m,h),(h,l),(l,h),(m,m) are kept; dropped terms are
            # <= 2^-24 relative.
            def split3(x, pfx):
                parts = []
                cur = x
                for lvl in range(3):
                    pb = io.tile([P2, HL], BF16, tag=f"{pfx}_b{lvl}")
                    nc.vector.tensor_copy(pb[:], cur[:])
                    parts.append(pb)
                    if lvl < 2:
                        res = io.tile([P2, HL], F32, tag=f"{pfx}_r{lvl}")
                        nc.vector.tensor_sub(res[:], cur[:], pb[:])
                        cur = res
                return parts  # [h, m, l] bf16 tiles, stacked hi|lo

            r_sp = split3(st_r, "rsp")
            i_sp = split3(st_i, "isp")
            # Views: top rows = hi-half splits, bottom rows = lo-half splits.
            hr = [p[0:BC] for p in r_sp]
            hh = [p[0:BC] for p in i_sp]
            lr = [p[BC:P2] for p in r_sp]
            ll = [p[BC:P2] for p in i_sp]
            # Negated lo-imag splits; compute in the bottom partition group so
            # DVE in/out partition bases match.
            nll = []
            for lvl in range(3):
                t = io.tile([P2, HL], BF16, tag=f"nll_b{lvl}")
                nc.vector.tensor_scalar_mul(t[BC:P2, :], i_sp[lvl][BC:P2, :], -1.0)
                nll.append(t[BC:P2])

            # Term pairing (a, b): lhsT row holds hi-part a, rhs row holds
            # lo-part b. Same lhsT rows serve real (even cols) and imag (odd).
            PAIRS = [(0, 0), (0, 1), (1, 0), (0, 2), (2, 0), (1, 1)]
            K = 2 * len(PAIRS)  # 12

            # lhsT rows, flattened batch-major: rows 0..5 = hr[a_k], 6..11 = hh[a_k]
            LH = io.tile([K, BC * HL], BF16, tag="LH")
            dma_engs = [nc.sync, nc.scalar]
            for k, (a, _) in enumerate(PAIRS):
                dma_engs[k % 2].dma_start(LH[k : k + 1, :], hr[a])
                dma_engs[(k + 1) % 2].dma_start(LH[6 + k : 7 + k, :], hh[a])

            # Interleaved rhs patterns, built batch-on-partitions then
            # flattened. PT1[b] = interleave(lr_b, ll_b)  (rows 0..5),
            # PT2[b] = interleave(-ll_b, lr_b)            (rows 6..11).
            # Built in the bottom partition group (rows BC..) so DVE in/out
            # partition bases match the lo-half source views.
            PT1 = []
            PT2 = []
            for lvl in range(3):
                t1 = io.tile([P2, 2 * HL], BF16, tag=f"PT1_{lvl}")
                v1 = t1[BC:P2, :].rearrange("p (j t) -> p j t", t=2)
                nc.vector.tensor_copy(v1[:, :, 0], lr[lvl])
                nc.vector.tensor_copy(v1[:, :, 1], ll[lvl])
                PT1.append(t1[BC:P2, :])
                t2 = io.tile([P2, 2 * HL], BF16, tag=f"PT2_{lvl}")
                v2 = t2[BC:P2, :].rearrange("p (j t) -> p j t", t=2)
                nc.vector.tensor_copy(v2[:, :, 0], nll[lvl])
                nc.vector.tensor_copy(v2[:, :, 1], lr[lvl])
                PT2.append(t2[BC:P2, :])
            RH = io.tile([K, BC * 2 * HL], BF16, tag="RH")
            for k, (_, b) in enumerate(PAIRS):
                dma_engs[k % 2].dma_start(RH[k : k + 1, :], PT1[b])
                dma_engs[(k + 1) % 2].dma_start(RH[6 + k : 7 + k, :], PT2[b])

            # out[b, ck*128+p, :] = hi[b, ck*128+p] * lo[b, :] as a K=12 matmul.
            for bi in range(BC):
                for ck in range(2):
                    acc = psum.tile([128, 512], F32, tag="acc", bufs=7)
                    lh_off = bi * HL + ck * 128
                    rh_off = bi * 2 * HL
                    nc.tensor.matmul(
                        acc[:],
                        LH[:, lh_off : lh_off + 128],
                        RH[:, rh_off : rh_off + 2 * HL],
                        start=True,
                        stop=True,
                    )
                    ot = stage.tile([128, 512], F32, tag="ot")
                    it = bi * 2 + ck
                    if it % 8 in (0, 3, 6):
                        nc.scalar.copy(ot[:], acc[:])
                    else:
                        nc.vector.tensor_copy(ot[:], acc[:])
                    out_eng = (nc.sync, nc.sync, nc.sync, nc.scalar)[it % 4]
                    out_eng.dma_start(out_d[bi, ck], ot[:])
    _legalize_single_wait(nc)
    return nc


_nc_cache = None


def _get_nc():
    global _nc_cache
    if _nc_cache is None:
        _nc_cache = build_bass()
    return _nc_cache


def run(ry_angles, rz_angles, trace=False):
    """Shard over 8 cores, run, gather. Returns (out [B, 2**Q] c64, results)."""
    ry = np.ascontiguousarray(np.asarray(ry_angles, dtype=np.float32))
    rz = np.ascontiguousarray(np.asarray(rz_angles, dtype=np.float32))
    assert ry.shape == (B, Q) and rz.shape == (B, Q)
    nc = _get_nc()
    in_maps = [
        {
            "ry": np.ascontiguousarray(ry[k * BC : (k + 1) * BC]),
            "rz": np.ascontiguousarray(rz[k * BC : (k + 1) * BC]),
        }
        for k in range(N_CORES)
    ]
    res = run_bass_kernel_spmd(nc, in_maps, list(range(N_CORES)), trace=trace)
    parts = [
        np.ascontiguousarray(r["out"]).reshape(BC, 2 * (1 << Q)).view(np.complex64)
        for r in res.results
    ]
    return np.concatenate(parts, axis=0), res


def kernel(ry_angles, rz_angles):
    out, _ = run(ry_angles, rz_angles, trace=False)
    return out



# revision 17
# speedup vs baseline: 1.1452x; 1.1452x over previous
"""Quantum angle-encoder state-vector kernel for Trainium2 (8 NeuronCores).

For each batch row b and qubit q the gate rz*ry applied to |0> contributes a
2-vector col0 = cos(ry/2)e^{-i rz/2}, col1 = sin(ry/2)e^{+i rz/2}; the output
state is the Kronecker product over 16 qubits (qubit 0 = MSB), [B, 2^16] c64.

Per core (32 batch rows, pure data parallel over 8 cores):
  * v = v_hi (x) v_lo with v_hi/v_lo the 8-qubit half-products (length 256),
    both built in POLAR form stacked on 64 partitions:
      - phases are additive -> ONE K=16 TensorE matmul against a constant
        0/1 selection matrix computes all 256 phase sums per row;
      - magnitudes multiply -> 3 DVE ops forming a doubling tree with
        stride-0 (broadcast) access patterns;
      - range-reduce theta into the Sin LUT domain with a single
        (theta + 65*pi) mod 2*pi tensor_scalar op per trig block.
  * Factors are 2-term bf16 splits (h + l, ~2^-16 exact); the 256x256 outer
    product is a K=8 bf16 matmul per (b, i-chunk) covering all (a,b) split
    pairs; rhs columns pre-interleaved so PSUM lands in complex memory order.
  * PSUM -> SBUF copies downcast to fp16 (tolerance 2e-2 >> fp16's 2^-11),
    4 PSUM banks per copy, alternating ScalarE/VectorE; SBUF -> HBM in
    512 KiB DMAs alternating the SP HWDGE ring and the GpSimd SWDGE ring.
    Host upcasts fp16 -> complex64 (free; only HW time is graded).

Notes for this toolchain: walrus encodes at most ONE semaphore wait per
instruction -- _legalize_single_wait() hoists extra Tile-emitted waits into
standalone EventSemaphore instructions.
"""

import numpy as np

import concourse.bass as bass
import concourse.mybir as mybir
import concourse.tile as tile
from concourse.bass_utils import run_bass_kernel_spmd

N_CORES = 8
B, Q = 256, 16
BC = B // N_CORES  # batch rows per core
HQ = Q // 2  # qubits per half
HL = 1 << HQ  # 256: length of each half-product
F32 = mybir.dt.float32
F16 = mybir.dt.float16
BF16 = mybir.dt.bfloat16
PI = float(np.pi)
PI_HALF = float(np.pi / 2)
TWO_PI = float(np.float32(2.0 * np.pi))
C_SIN = float(np.float32(65.0 * np.pi))
C_COS = float(np.float32(65.0 * np.pi + np.pi / 2))
SCL = 1.0 - 1e-5  # keep sin argument strictly inside [-pi, pi]

_AF = mybir.ActivationFunctionType
_OP = mybir.AluOpType

N_CHUNK = 2 * BC  # 64 output chunks of [128, 512] f32 per core
G_CHUNK = 4  # chunks per PSUM drain group
N_GROUP = N_CHUNK // G_CHUNK  # 16


def _legalize_single_wait(nc):
    """This walrus build encodes at most one semaphore wait per instruction
    ("Too many sync wait commands" otherwise). Hoist extra waits into
    standalone EventSemaphore instructions placed immediately before — a
    sequencer-level wait gates everything after it on the same engine, so
    semantics are preserved (slightly stronger ordering)."""
    cnt = 0
    for fn in nc.m.functions:
        for blk in fn.blocks:
            out = []
            for ins in blk.instructions:
                si = ins.sync_info
                if si is not None and si.on_wait is not None and len(si.on_wait) > 1:
                    waits = list(si.on_wait)
                    for w in waits[:-1]:
                        cnt += 1
                        ev = mybir.InstEventSemaphore(
                            name=f"{ins.name}-presync-{cnt}", ins=[], outs=[]
                        )
                        ev.engine = ins.engine
                        ev.sync_info = mybir.SyncInfo(on_wait=[w], on_update=[])
                        out.append(ev)
                    ins.sync_info = mybir.SyncInfo(
                        on_wait=[waits[-1]], on_update=list(si.on_update)
                    )
                out.append(ins)
            try:
                blk.instructions = out
            except Exception:
                blk.instructions[:] = out
    return cnt


def build_bass(legalize=True):
    nc = bass.Bass()
    ry_d = nc.dram_tensor("ry", [BC, Q], F32, kind="ExternalInput")
    rz_d = nc.dram_tensor("rz", [BC, Q], F32, kind="ExternalInput")
    # group g holds chunks 4g..4g+3; chunk c=(bi,ck) = [128, 512] fp16 values
    out_d = nc.dram_tensor("out", [N_GROUP, 128, G_CHUNK * 512], F16,
                           kind="ExternalOutput")

    ident_np = np.eye(2 * BC, dtype=np.float32)
    ident_d = nc.inline_tensor(ident_np, name="ident_const")
    sel_np = np.zeros((2 * HQ, HL), dtype=np.float32)
    for q in range(HQ):
        for t in range(2):
            bits = (np.arange(HL) >> (HQ - 1 - q)) & 1
            sel_np[t * HQ + q, :] = (bits == t).astype(np.float32)
    sel_d = nc.inline_tensor(sel_np, name="sel_const")

    with tile.TileContext(nc) as tc:
        with (
            tc.tile_pool(name="io", bufs=1) as io,
            tc.tile_pool(name="stage", bufs=3) as stage,
            tc.tile_pool(name="psum", bufs=2, space="PSUM") as psum,
        ):
            P2 = 2 * BC
            # Stacked angle layout [2*BC, HQ]: rows 0..31 = qubits 0..7 (hi
            # half), rows 32..63 = qubits 8..15 (lo half), same batch rows.
            sry = io.tile([P2, HQ], F32, tag="sry")
            srz = io.tile([P2, HQ], F32, tag="srz")
            nc.sync.dma_start(sry[0:BC, :], ry_d[:, 0:HQ])
            nc.scalar.dma_start(sry[BC:P2, :], ry_d[:, HQ:Q])
            nc.sync.dma_start(srz[0:BC, :], rz_d[:, 0:HQ])
            nc.scalar.dma_start(srz[BC:P2, :], rz_d[:, HQ:Q])
            ident = io.tile([P2, P2], F32, tag="ident")
            nc.sync.dma_start(ident[:], ident_d[:])
            sel = io.tile([2 * HQ, HL], F32, tag="sel")
            nc.sync.dma_start(sel[:], sel_d[:])

            # Per-qubit columns in polar form:
            #   col0 = cos(ry/2) e^{-i rz/2}: mag |cos|, phase -rz/2 + pi[c<0]
            #   col1 = sin(ry/2) e^{+i rz/2}: mag |sin|, phase +rz/2 + pi[s<0]
            pih = io.tile([P2, 1], F32, tag="pih")
            nc.vector.memset(pih[:], PI_HALF)
            c = io.tile([P2, HQ], F32, tag="c")
            s = io.tile([P2, HQ], F32, tag="s")
            nc.scalar.activation(c[:], sry[:], _AF.Sin, bias=pih[:], scale=0.5)
            nc.scalar.activation(s[:], sry[:], _AF.Sin, scale=0.5)
            M = io.tile([P2, 2 * HQ], F32, tag="M")  # col t*8+q = mag_t[q]
            nc.scalar.activation(M[:, 0:HQ], c[:], _AF.Abs)
            nc.scalar.activation(M[:, HQ : 2 * HQ], s[:], _AF.Abs)
            hrz = io.tile([P2, HQ], F32, tag="hrz")
            nc.vector.tensor_scalar_mul(hrz[:], srz[:], 0.5)
            mkc = io.tile([P2, HQ], F32, tag="mkc")
            mks = io.tile([P2, HQ], F32, tag="mks")
            nc.vector.tensor_scalar(mkc[:], c[:], 0.0, None, op0=_OP.is_lt)
            nc.vector.tensor_scalar(mks[:], s[:], 0.0, None, op0=_OP.is_lt)
            PHI = io.tile([P2, 2 * HQ], F32, tag="PHI")
            nc.vector.scalar_tensor_tensor(
                PHI[:, 0:HQ], mkc[:], PI, hrz[:], op0=_OP.mult, op1=_OP.subtract
            )
            nc.vector.scalar_tensor_tensor(
                PHI[:, HQ : 2 * HQ], mks[:], PI, hrz[:], op0=_OP.mult, op1=_OP.add
            )

            # theta[b, i] = sum_q PHI[b, bit_q(i)*8 + q] via transpose+matmul.
            tp = psum.tile([2 * HQ, P2], F32, tag="acc")
            nc.tensor.transpose(tp[:], PHI[:], ident[:])
            vals = io.tile([2 * HQ, P2], F32, tag="vals")
            nc.vector.tensor_copy(vals[:], tp[:])
            theta = psum.tile([P2, HL], F32, tag="acc")
            nc.tensor.matmul(theta[:], vals[:], sel[:], start=True, stop=True)

            # Magnitude doubling tree: 3 DVE ops with stride-0 broadcasts.
            # T1[p, pr, b0, b1] = M[p, b0*8+2pr] * M[p, b1*8+2pr+1]
            T1 = io.tile([P2, 16], F32, tag="T1")
            o1 = T1[:, :].rearrange("p (pr b0 b1) -> p pr b0 b1", pr=4, b0=2, b1=2)
            v0 = M[:, :].rearrange("p (b pr x) -> p pr b x", b=2, pr=4, x=2)
            in0 = v0[:, :, :, 0:1].broadcast_to([P2, 4, 2, 2])
            v1 = M[:, :].rearrange("p (b pr x) -> p pr x b", b=2, pr=4, x=2)
            in1 = v1[:, :, 1:2, :].broadcast_to([P2, 4, 2, 2])
            nc.vector.tensor_tensor(o1, in0, in1, op=_OP.mult)
            # T2[p, h, a, b] = T1[p, h*8+a] * T1[p, h*8+4+b]
            T2 = io.tile([P2, 32], F32, tag="T2")
            o2 = T2[:, :].rearrange("p (h a b) -> p h a b", h=2, a=4, b=4)
            w0 = T1[:, :].rearrange("p (h x a) -> p h a x", h=2, x=2, a=4)
            i20 = w0[:, :, :, 0:1].broadcast_to([P2, 2, 4, 4])
            w1 = T1[:, :].rearrange("p (h x b) -> p h x b", h=2, x=2, b=4)
            i21 = w1[:, :, 1:2, :].broadcast_to([P2, 2, 4, 4])
            nc.vector.tensor_tensor(o2, i20, i21, op=_OP.mult)
            # m[p, a*16+b] = T2[p, a] * T2[p, 16+b]
            m = io.tile([P2, HL], F32, tag="m")
            om = m[:, :].rearrange("p (a b) -> p a b", a=16, b=16)
            im0 = T2[:, 0:16].unsqueeze(2).broadcast_to([P2, 16, 16])
            im1 = T2[:, 16:32].unsqueeze(1).broadcast_to([P2, 16, 16])
            nc.vector.tensor_tensor(om, im0, im1, op=_OP.mult)

            # Range-reduce theta into [-pi, pi]: k = round(theta/2pi) via the
            # f32->i32->f32 nearest-rounding cast, then a fused 3-term
            # Cody-Waite cascade; cos block = sin block shifted +pi/2 with a
            # single-period wrap. S = sin(SCL*(Y)) with SCL guarding the LUT
            # domain edge.
            INV2PI = float(1.0 / (2.0 * np.pi))
            CW1 = 6.28125  # 9 mantissa bits: k*CW1 exact for k <= 2^14
            CW2 = float(np.float32(2.0 * np.pi - CW1))
            CW3 = float(2.0 * np.pi - CW1 - np.float64(np.float32(CW2)))
            # k = round(theta/2pi) via the magic-constant trick: adding
            # 1.5*2^23 forces fp32 round-to-nearest-integer (IEEE, identical
            # on DVE and in sim); subtracting it back yields the integer.
            MAGIC = float(1.5 * 2.0**23)
            t1 = io.tile([P2, HL], F32, tag="t1")
            nc.vector.tensor_scalar(
                t1[:], theta[:], INV2PI, MAGIC, op0=_OP.mult, op1=_OP.add
            )
            nf = io.tile([P2, HL], F32, tag="nf")
            nc.vector.tensor_scalar(nf[:], t1[:], MAGIC, None, op0=_OP.subtract)
            Y = io.tile([P2, 2 * HL], F32, tag="Y")
            yr = io.tile([P2, HL], F32, tag="yr")
            nc.vector.scalar_tensor_tensor(
                yr[:], nf[:], -CW1, theta[:], op0=_OP.mult, op1=_OP.add
            )
            nc.vector.scalar_tensor_tensor(
                Y[:, 0:HL], nf[:], -CW2, yr[:], op0=_OP.mult, op1=_OP.add
            )
            # cos block: red + pi/2, wrapped by one period where red > pi/2
            # (the +pi/2 itself rides in the Sin bias below)
            msk = io.tile([P2, HL], F32, tag="msk")
            nc.vector.tensor_scalar(msk[:], Y[:, 0:HL], PI_HALF, None, op0=_OP.is_gt)
            nc.vector.scalar_tensor_tensor(
                Y[:, HL : 2 * HL], msk[:], -2.0 * PI, Y[:, 0:HL],
                op0=_OP.mult, op1=_OP.add,
            )
            S = io.tile([P2, 2 * HL], F32, tag="S")
            sphb = io.tile([P2, 1], F32, tag="sphb")
            nc.vector.memset(sphb[:], SCL * PI_HALF)
            nc.scalar.activation(S[:, 0:HL], Y[:, 0:HL], _AF.Sin, scale=SCL)
            nc.scalar.activation(
                S[:, HL : 2 * HL], Y[:, HL : 2 * HL], _AF.Sin, bias=sphb[:], scale=SCL
            )
            sin_a = S[:, 0:HL]
            cos_a = S[:, HL : 2 * HL]

            # ---- hi half: HS [32, 2048] bf16 = [hr0 hr1 hh0 hh1] x2 ----
            vrh = io.tile([BC, HL], F32, tag="vrh")
            vih = io.tile([BC, HL], F32, tag="vih")
            nc.vector.tensor_mul(vrh[:], m[0:BC, :], cos_a[0:BC, :])
            nc.gpsimd.tensor_mul(vih[:], m[0:BC, :], sin_a[0:BC, :])
            HS = io.tile([BC, 4 * HL], BF16, tag="HS")  # [hr0 hr1 hh0 hh1]
            rr = io.tile([BC, HL], F32, tag="rr")
            ri = io.tile([BC, HL], F32, tag="ri")
            nc.scalar.copy(HS[:, 0:HL], vrh[:])
            nc.vector.tensor_sub(rr[:], vrh[:], HS[:, 0:HL])
            nc.scalar.copy(HS[:, HL : 2 * HL], rr[:])
            nc.scalar.copy(HS[:, 2 * HL : 3 * HL], vih[:])
            nc.vector.tensor_sub(ri[:], vih[:], HS[:, 2 * HL : 3 * HL])
            nc.scalar.copy(HS[:, 3 * HL : 4 * HL], ri[:])

            # ---- lo half (partitions 32:64): interleaved patterns ----
            # PTf[:, 0:512] = PT1f = (lr, ll) interleaved; [:, 512:1024] = PT2f
            PTf = io.tile([P2, 4 * HL], F32, tag="PTf")
            p1 = PTf[BC:P2, 0 : 2 * HL].rearrange("p (j t) -> p j t", t=2)
            p2 = PTf[BC:P2, 2 * HL : 4 * HL].rearrange("p (j t) -> p j t", t=2)
            nc.vector.tensor_mul(p1[:, :, 0], m[BC:P2, :], cos_a[BC:P2, :])
            nc.vector.tensor_mul(p1[:, :, 1], m[BC:P2, :], sin_a[BC:P2, :])
            nc.vector.scalar_tensor_tensor(
                p2[:, :, 0], sin_a[BC:P2, :], -1.0, m[BC:P2, :],
                op0=_OP.mult, op1=_OP.mult,
            )
            nc.vector.tensor_mul(p2[:, :, 1], m[BC:P2, :], cos_a[BC:P2, :])
            # PTT blocks (512 bf16 each): [PT1_0, PT2_0, PT1_1, PT2_1]
            PTT = io.tile([P2, 8 * HL], BF16, tag="PTT")
            r1 = io.tile([P2, 2 * HL], F32, tag="r1")
            r2 = io.tile([P2, 2 * HL], F32, tag="r2")
            nc.scalar.copy(PTT[BC:P2, 0 : 2 * HL], PTf[BC:P2, 0 : 2 * HL])
            nc.vector.tensor_sub(
                r1[BC:P2, :], PTf[BC:P2, 0 : 2 * HL], PTT[BC:P2, 0 : 2 * HL]
            )
            nc.scalar.copy(PTT[BC:P2, 4 * HL : 6 * HL], r1[BC:P2, :])
            nc.scalar.copy(PTT[BC:P2, 2 * HL : 4 * HL], PTf[BC:P2, 2 * HL : 4 * HL])
            nc.vector.tensor_sub(
                r2[BC:P2, :], PTf[BC:P2, 2 * HL : 4 * HL], PTT[BC:P2, 2 * HL : 4 * HL]
            )
            nc.scalar.copy(PTT[BC:P2, 6 * HL : 8 * HL], r2[BC:P2, :])

            # PE warm-up: keep HAM at K=8/8 through the gather window. Reads
            # HS so these land after the first hi-split cast, right before
            # the main loop.
            warm_ps = psum.tile([128, 512], F32, tag="acc")
            for w in range(10):
                nc.tensor.matmul(
                    warm_ps[:, :],
                    HS[0:8, 0:128],
                    HS[0:8, 0:512],
                    start=True,
                    stop=True,
                )

            # ---- gathers: [K, flat] layouts for the outer-product matmuls ----
            # Row k = (b, c, a): lhs part = {c=0: hr_a, c=1: hh_a} (HS block
            # c*2+a), rhs part = PT{c}_b (PTT block b*2+c). Per-row DMAs
            # collapse 32 partitions into one row; spread over 3 queues.
            LH = io.tile([8, BC * HL], BF16, tag="LH")
            RH = io.tile([8, BC * 2 * HL], BF16, tag="RH")
            dma_engs = [nc.sync, nc.scalar, nc.gpsimd]
            for k in range(8):
                b, c, a = (k >> 2) & 1, (k >> 1) & 1, k & 1
                hs_blk = c * 2 + a
                pt_blk = b * 2 + c
                dma_engs[(2 * k) % 3].dma_start(
                    LH[k : k + 1, :], HS[:, hs_blk * HL : (hs_blk + 1) * HL]
                )
                dma_engs[(2 * k + 1) % 3].dma_start(
                    RH[k : k + 1, :],
                    PTT[BC:P2, pt_blk * 2 * HL : (pt_blk + 1) * 2 * HL],
                )

            # ---- main loop: 16 groups x (4 matmuls + 1 copy + 1 DMA) ----
            for g in range(N_GROUP):
                acc = psum.tile([128, G_CHUNK * 512], F32, tag="acc")
                for t in range(G_CHUNK):
                    ch = g * G_CHUNK + t
                    bi, ck = ch >> 1, ch & 1
                    lo = bi * HL + ck * 128
                    nc.tensor.matmul(
                        acc[:, t * 512 : (t + 1) * 512],
                        LH[:, lo : lo + 128],
                        RH[:, bi * 2 * HL : (bi + 1) * 2 * HL],
                        start=True,
                        stop=True,
                    )
                st = stage.tile([128, G_CHUNK * 512], F16, tag="st")
                # DVE is slower per element; give it the minority share.
                if g % 3 == 1:
                    nc.vector.tensor_copy(st[:], acc[:])
                else:
                    nc.scalar.copy(st[:], acc[:])
                out_eng = nc.sync if g % 2 == 0 else nc.gpsimd
                out_eng.dma_start(out_d[g], st[:])
    if legalize:
        _legalize_single_wait(nc)
    return nc


_nc_cache = None


def _get_nc():
    global _nc_cache
    if _nc_cache is None:
        _nc_cache = build_bass()
    return _nc_cache


def run(ry_angles, rz_angles, trace=False):
    """Shard over 8 cores, run, gather. Returns (out [B, 2**Q] c64, results)."""
    ry = np.ascontiguousarray(np.asarray(ry_angles, dtype=np.float32))
    rz = np.ascontiguousarray(np.asarray(rz_angles, dtype=np.float32))
    assert ry.shape == (B, Q) and rz.shape == (B, Q)
    nc = _get_nc()
    in_maps = [
        {
            "ry": np.ascontiguousarray(ry[k * BC : (k + 1) * BC]),
            "rz": np.ascontiguousarray(rz[k * BC : (k + 1) * BC]),
        }
        for k in range(N_CORES)
    ]
    res = run_bass_kernel_spmd(nc, in_maps, list(range(N_CORES)), trace=trace)
    parts = []
    for r in res.results:
        a = np.ascontiguousarray(r["out"])  # [16, 128, 2048] fp16
        a = a.reshape(N_GROUP, 128, G_CHUNK, 512).transpose(0, 2, 1, 3)
        a = a.reshape(BC, 2, 128, 512).astype(np.float32)
        parts.append(a.reshape(BC, 2 * (1 << Q)).view(np.complex64))
    return np.concatenate(parts, axis=0), res


def kernel(ry_angles, rz_angles):
    out, _ = run(ry_angles, rz_angles, trace=False)
    return out


# revision 20
# speedup vs baseline: 1.5464x; 1.3503x over previous
"""Quantum angle-encoder state-vector kernel for Trainium2 (8 NeuronCores).

For each batch row b and qubit q the gate rz*ry applied to |0> contributes a
2-vector col0 = cos(ry/2)e^{-i rz/2}, col1 = sin(ry/2)e^{+i rz/2}; the output
state is the Kronecker product over 16 qubits (qubit 0 = MSB), [B, 2^16] c64.

Per core (32 batch rows, pure data parallel over 8 cores):
  * v = v_hi (x) v_lo with v_hi/v_lo the 8-qubit half-products (length 256),
    both built in POLAR form stacked on 64 partitions:
      - phases are additive -> ONE K=16 TensorE matmul against a constant
        0/1 selection matrix computes all 256 phase sums per row;
      - magnitudes multiply -> 3 DVE ops forming a doubling tree with
        stride-0 (broadcast) access patterns;
      - range-reduce theta into [-pi, pi] (Sin LUT domain) with the
        1.5*2^23 magic-constant round + a 2-term Cody-Waite cascade; the
        cos block is the sin block wrapped by one period past pi/2.
  * Factors are rounded once to bf16 (rel err ~0.5% << the 2e-2 gate, and
    PE is stuck at the cold 1.2 GHz clock so extra split terms buy nothing);
    the 256x256 outer product is a K=2 bf16 matmul per (b, i-chunk), rhs
    pre-interleaved so PSUM lands in complex memory order. lhsT/rhs live in
    two partition groups (0 and 32) and chunks alternate between them so
    LDWEIGHTS (other row group) overlaps the running matmul.
  * PSUM -> SBUF copies downcast to fp16 (tolerance allows it; host upcasts
    for free -- only HW time is graded), 2 banks per copy, alternating
    VectorE/ScalarE; SBUF -> HBM in 512 KiB DMAs alternating the SP HWDGE
    ring and the GpSimd SWDGE ring (SWDGE only for big output transfers --
    its per-descriptor Q7 cost makes it terrible for strided gathers).

Notes for this toolchain: walrus encodes at most ONE semaphore wait per
instruction -- _legalize_single_wait() hoists extra Tile-emitted waits into
standalone EventSemaphore instructions.
"""

import numpy as np

import concourse.bass as bass
import concourse.mybir as mybir
import concourse.tile as tile
from concourse.bass_utils import run_bass_kernel_spmd

N_CORES = 8
B, Q = 256, 16
BC = B // N_CORES  # batch rows per core
HQ = Q // 2  # qubits per half
HL = 1 << HQ  # 256: length of each half-product
F32 = mybir.dt.float32
F16 = mybir.dt.float16
BF16 = mybir.dt.bfloat16
PI = float(np.pi)
PI_HALF = float(np.pi / 2)
SCL = 1.0 - 1e-5  # keep sin argument strictly inside [-pi, pi]

_AF = mybir.ActivationFunctionType
_OP = mybir.AluOpType

N_CHUNK = 2 * BC  # 64 output chunks of [128, 512] f32 values per core
N_DMAG = 16  # output DMA groups (4 chunks = 512 KiB each)


def _legalize_single_wait(nc):
    """This walrus build encodes at most one semaphore wait per instruction
    ("Too many sync wait commands" otherwise). Hoist extra waits into
    standalone EventSemaphore instructions placed immediately before — a
    sequencer-level wait gates everything after it on the same engine, so
    semantics are preserved (slightly stronger ordering)."""
    cnt = 0
    for fn in nc.m.functions:
        for blk in fn.blocks:
            out = []
            for ins in blk.instructions:
                si = ins.sync_info
                if si is not None and si.on_wait is not None and len(si.on_wait) > 1:
                    waits = list(si.on_wait)
                    for w in waits[:-1]:
                        cnt += 1
                        ev = mybir.InstEventSemaphore(
                            name=f"{ins.name}-presync-{cnt}", ins=[], outs=[]
                        )
                        ev.engine = ins.engine
                        ev.sync_info = mybir.SyncInfo(on_wait=[w], on_update=[])
                        out.append(ev)
                    ins.sync_info = mybir.SyncInfo(
                        on_wait=[waits[-1]], on_update=list(si.on_update)
                    )
                out.append(ins)
            try:
                blk.instructions = out
            except Exception:
                blk.instructions[:] = out
    return cnt


def build_bass(legalize=True):
    nc = bass.Bass()
    ry_d = nc.dram_tensor("ry", [BC, Q], F32, kind="ExternalInput")
    rz_d = nc.dram_tensor("rz", [BC, Q], F32, kind="ExternalInput")
    # group g holds chunks 4g..4g+3; chunk c=(bi,ck) = [128, 512] fp16 values
    out_d = nc.dram_tensor("out", [N_DMAG, 128, 2048], F16, kind="ExternalOutput")

    ident_np = np.eye(2 * BC, dtype=np.float32)
    ident_d = nc.inline_tensor(ident_np, name="ident_const")
    sel_np = np.zeros((2 * HQ, HL), dtype=np.float32)
    for q in range(HQ):
        for t in range(2):
            bits = (np.arange(HL) >> (HQ - 1 - q)) & 1
            sel_np[t * HQ + q, :] = (bits == t).astype(np.float32)
    sel_d = nc.inline_tensor(sel_np, name="sel_const")

    with tile.TileContext(nc) as tc:
        with (
            tc.tile_pool(name="io", bufs=1) as io,
            tc.tile_pool(name="stage", bufs=3) as stage,
            tc.tile_pool(name="psum", bufs=3, space="PSUM") as psum,
        ):
            P2 = 2 * BC
            # Stacked angle layout [2*BC, HQ]: rows 0..31 = qubits 0..7 (hi
            # half), rows 32..63 = qubits 8..15 (lo half), same batch rows.
            # One DMA per input: dram side iterates (h, b, q).
            sry = io.tile([P2, HQ], F32, tag="sry")
            srz = io.tile([P2, HQ], F32, tag="srz")
            nc.sync.dma_start(sry[0:BC, :], ry_d[:, 0:HQ])
            nc.scalar.dma_start(sry[BC:P2, :], ry_d[:, HQ:Q])
            nc.sync.dma_start(srz[0:BC, :], rz_d[:, 0:HQ])
            nc.scalar.dma_start(srz[BC:P2, :], rz_d[:, HQ:Q])
            ident = io.tile([P2, P2], F32, tag="ident")
            nc.sync.dma_start(ident[:], ident_d[:])
            sel = io.tile([2 * HQ, HL], F32, tag="sel")
            nc.sync.dma_start(sel[:], sel_d[:])

            # Per-qubit columns in polar form:
            #   col0 = cos(ry/2) e^{-i rz/2}: mag |cos|, phase -rz/2 + pi[c<0]
            #   col1 = sin(ry/2) e^{+i rz/2}: mag |sin|, phase +rz/2 + pi[s<0]
            pih = io.tile([P2, 1], F32, tag="pih")
            nc.vector.memset(pih[:], PI_HALF)
            sphb = io.tile([P2, 1], F32, tag="sphb")
            nc.vector.memset(sphb[:], SCL * PI_HALF)
            c = io.tile([P2, HQ], F32, tag="c")
            s = io.tile([P2, HQ], F32, tag="s")
            nc.scalar.activation(c[:], sry[:], _AF.Sin, bias=pih[:], scale=0.5)
            nc.scalar.activation(s[:], sry[:], _AF.Sin, scale=0.5)
            M = io.tile([P2, 2 * HQ], F32, tag="M")  # col t*8+q = mag_t[q]
            nc.scalar.activation(M[:, 0:HQ], c[:], _AF.Abs)
            nc.scalar.activation(M[:, HQ : 2 * HQ], s[:], _AF.Abs)
            hrz = io.tile([P2, HQ], F32, tag="hrz")
            nc.vector.tensor_scalar_mul(hrz[:], srz[:], 0.5)
            mkc = io.tile([P2, HQ], F32, tag="mkc")
            mks = io.tile([P2, HQ], F32, tag="mks")
            nc.vector.tensor_scalar(mkc[:], c[:], 0.0, None, op0=_OP.is_lt)
            nc.vector.tensor_scalar(mks[:], s[:], 0.0, None, op0=_OP.is_lt)
            PHI = io.tile([P2, 2 * HQ], F32, tag="PHI")
            nc.vector.scalar_tensor_tensor(
                PHI[:, 0:HQ], mkc[:], PI, hrz[:], op0=_OP.mult, op1=_OP.subtract
            )
            nc.vector.scalar_tensor_tensor(
                PHI[:, HQ : 2 * HQ], mks[:], PI, hrz[:], op0=_OP.mult, op1=_OP.add
            )

            # theta[b, i] = sum_q PHI[b, bit_q(i)*8 + q] via transpose+matmul.
            tp = psum.tile([2 * HQ, P2], F32, tag="acc")
            nc.tensor.transpose(tp[:], PHI[:], ident[:])
            vals = io.tile([2 * HQ, P2], F32, tag="vals")
            nc.vector.tensor_copy(vals[:], tp[:])
            theta = psum.tile([P2, HL], F32, tag="acc")
            nc.tensor.matmul(theta[:], vals[:], sel[:], start=True, stop=True)

            # Magnitude doubling tree: 3 DVE ops with stride-0 broadcasts.
            # T1[p, pr, b0, b1] = M[p, b0*8+2pr] * M[p, b1*8+2pr+1]
            T1 = io.tile([P2, 16], F32, tag="T1")
            o1 = T1[:, :].rearrange("p (pr b0 b1) -> p pr b0 b1", pr=4, b0=2, b1=2)
            v0 = M[:, :].rearrange("p (b pr x) -> p pr b x", b=2, pr=4, x=2)
            in0 = v0[:, :, :, 0:1].broadcast_to([P2, 4, 2, 2])
            v1 = M[:, :].rearrange("p (b pr x) -> p pr x b", b=2, pr=4, x=2)
            in1 = v1[:, :, 1:2, :].broadcast_to([P2, 4, 2, 2])
            nc.vector.tensor_tensor(o1, in0, in1, op=_OP.mult)
            # T2[p, h, a, b] = T1[p, h*8+a] * T1[p, h*8+4+b]
            T2 = io.tile([P2, 32], F32, tag="T2")
            o2 = T2[:, :].rearrange("p (h a b) -> p h a b", h=2, a=4, b=4)
            w0 = T1[:, :].rearrange("p (h x a) -> p h a x", h=2, x=2, a=4)
            i20 = w0[:, :, :, 0:1].broadcast_to([P2, 2, 4, 4])
            w1 = T1[:, :].rearrange("p (h x b) -> p h x b", h=2, x=2, b=4)
            i21 = w1[:, :, 1:2, :].broadcast_to([P2, 2, 4, 4])
            nc.vector.tensor_tensor(o2, i20, i21, op=_OP.mult)
            # m[p, a*16+b] = T2[p, a] * T2[p, 16+b]
            m = io.tile([P2, HL], F32, tag="m")
            om = m[:, :].rearrange("p (a b) -> p a b", a=16, b=16)
            im0 = T2[:, 0:16].unsqueeze(2).broadcast_to([P2, 16, 16])
            im1 = T2[:, 16:32].unsqueeze(1).broadcast_to([P2, 16, 16])
            nc.vector.tensor_tensor(om, im0, im1, op=_OP.mult)

            # Range-reduce theta into [-pi, pi]: k = round(theta/2pi) via the
            # magic-constant trick (1.5*2^23 forces round-to-nearest-integer,
            # IEEE-identical on DVE and in sim), then 2-term Cody-Waite.
            INV2PI = float(1.0 / (2.0 * np.pi))
            CW1 = 6.28125  # 9 mantissa bits: k*CW1 exact
            CW2 = float(np.float32(2.0 * np.pi - CW1))
            MAGIC = float(1.5 * 2.0**23)
            t1 = io.tile([P2, HL], F32, tag="t1")
            nc.vector.tensor_scalar(
                t1[:], theta[:], INV2PI, MAGIC, op0=_OP.mult, op1=_OP.add
            )
            nf = io.tile([P2, HL], F32, tag="nf")
            nc.vector.tensor_scalar(nf[:], t1[:], MAGIC, None, op0=_OP.subtract)
            Y = io.tile([P2, 2 * HL], F32, tag="Y")
            yr = io.tile([P2, HL], F32, tag="yr")
            nc.vector.scalar_tensor_tensor(
                yr[:], nf[:], -CW1, theta[:], op0=_OP.mult, op1=_OP.add
            )
            nc.vector.scalar_tensor_tensor(
                Y[:, 0:HL], nf[:], -CW2, yr[:], op0=_OP.mult, op1=_OP.add
            )
            # cos block: red + pi/2, wrapped one period where red > pi/2
            # (the +pi/2 itself rides in the Sin bias below)
            msk = io.tile([P2, HL], F32, tag="msk")
            nc.vector.tensor_scalar(msk[:], Y[:, 0:HL], PI_HALF, None, op0=_OP.is_gt)
            nc.vector.scalar_tensor_tensor(
                Y[:, HL : 2 * HL], msk[:], -2.0 * PI, Y[:, 0:HL],
                op0=_OP.mult, op1=_OP.add,
            )
            S = io.tile([P2, 2 * HL], F32, tag="S")
            nc.scalar.activation(S[:, 0:HL], Y[:, 0:HL], _AF.Sin, scale=SCL)
            nc.scalar.activation(
                S[:, HL : 2 * HL], Y[:, HL : 2 * HL], _AF.Sin, bias=sphb[:], scale=SCL
            )
            sin_a = S[:, 0:HL]
            cos_a = S[:, HL : 2 * HL]

            # Factors, rounded once to bf16 by the multiply itself.
            # hi half: HS = [hr | hh];  lo half (partitions 32:64): PTT =
            # [PT1 | PT2] with PT1 = (lr, ll) interleaved, PT2 = (-ll, lr).
            HS = io.tile([BC, 2 * HL], BF16, tag="HS")
            nc.vector.tensor_mul(HS[:, 0:HL], m[0:BC, :], cos_a[0:BC, :])
            nc.vector.tensor_mul(HS[:, HL : 2 * HL], m[0:BC, :], sin_a[0:BC, :])
            PTT = io.tile([P2, 4 * HL], BF16, tag="PTT")
            p1 = PTT[BC:P2, 0 : 2 * HL].rearrange("p (j t) -> p j t", t=2)
            p2 = PTT[BC:P2, 2 * HL : 4 * HL].rearrange("p (j t) -> p j t", t=2)
            nc.vector.tensor_mul(p1[:, :, 0], m[BC:P2, :], cos_a[BC:P2, :])
            nc.vector.tensor_mul(p1[:, :, 1], m[BC:P2, :], sin_a[BC:P2, :])
            nc.vector.scalar_tensor_tensor(
                p2[:, :, 0], sin_a[BC:P2, :], -1.0, m[BC:P2, :],
                op0=_OP.mult, op1=_OP.mult,
            )
            nc.vector.tensor_mul(p2[:, :, 1], m[BC:P2, :], cos_a[BC:P2, :])

            # Gathers into [K=2, flat] layouts, duplicated at partition
            # offset 32 so consecutive chunks alternate PE row groups.
            LHX = io.tile([34, BC * HL], BF16, tag="LHX")
            RHX = io.tile([34, BC * 2 * HL], BF16, tag="RHX")
            nc.sync.dma_start(LHX[0:1, :], HS[:, 0:HL])
            nc.scalar.dma_start(LHX[1:2, :], HS[:, HL : 2 * HL])
            nc.sync.dma_start(RHX[0:1, :], PTT[BC:P2, 0 : 2 * HL])
            nc.scalar.dma_start(RHX[1:2, :], PTT[BC:P2, 2 * HL : 4 * HL])
            nc.sync.dma_start(LHX[32:34, :], LHX[0:2, :])
            nc.scalar.dma_start(RHX[32:34, :], RHX[0:2, :])

            # ---- main loop: 32 groups x (2 matmuls + 1 copy); DMA per 2 ----
            st = None
            for g in range(N_CHUNK // 2):
                acc = psum.tile([128, 1024], F32, tag="acc")
                for t in range(2):
                    ch = g * 2 + t
                    bi, ck = ch >> 1, ch & 1
                    po = 32 * (ch & 1)
                    lo = bi * HL + ck * 128
                    nc.tensor.matmul(
                        acc[:, t * 512 : (t + 1) * 512],
                        LHX[po : po + 2, lo : lo + 128],
                        RHX[po : po + 2, bi * 2 * HL : (bi + 1) * 2 * HL],
                        start=True,
                        stop=True,
                    )
                if g % 2 == 0:
                    st = stage.tile([128, 2048], F16, tag="st")
                dst = st[:, (g % 2) * 1024 : (g % 2 + 1) * 1024]
                if g % 2 == 0:
                    nc.vector.tensor_copy(dst, acc[:])
                else:
                    nc.scalar.copy(dst, acc[:])
                if g % 2 == 1:
                    out_eng = nc.sync if (g // 2) % 2 == 0 else nc.gpsimd
                    out_eng.dma_start(out_d[g // 2], st[:])
    if legalize:
        _legalize_single_wait(nc)
    return nc


_nc_cache = None


def _get_nc():
    global _nc_cache
    if _nc_cache is None:
        _nc_cache = build_bass()
    return _nc_cache


def run(ry_angles, rz_angles, trace=False):
    """Shard over 8 cores, run, gather. Returns (out [B, 2**Q] c64, results)."""
    ry = np.ascontiguousarray(np.asarray(ry_angles, dtype=np.float32))
    rz = np.ascontiguousarray(np.asarray(rz_angles, dtype=np.float32))
    assert ry.shape == (B, Q) and rz.shape == (B, Q)
    nc = _get_nc()
    in_maps = [
        {
            "ry": np.ascontiguousarray(ry[k * BC : (k + 1) * BC]),
            "rz": np.ascontiguousarray(rz[k * BC : (k + 1) * BC]),
        }
        for k in range(N_CORES)
    ]
    res = run_bass_kernel_spmd(nc, in_maps, list(range(N_CORES)), trace=trace)
    parts = []
    for r in res.results:
        a = np.ascontiguousarray(r["out"])  # [16, 128, 2048] fp16
        a = a.reshape(N_DMAG, 128, 4, 512).transpose(0, 2, 1, 3)
        a = a.reshape(BC, 2, 128, 512).astype(np.float32)
        parts.append(a.reshape(BC, 2 * (1 << Q)).view(np.complex64))
    return np.concatenate(parts, axis=0), res


def kernel(ry_angles, rz_angles):
    out, _ = run(ry_angles, rz_angles, trace=False)
    return out
